# revision 37
# baseline (speedup 1.0000x reference)
"""4-bit group-quantized linear via fp8 DoubleRow matmul, column-parallel on 8 cores.

out = x @ W.T with W[n,k] = (q/15)*range[n,g] + min[n,g], groups of 512 k.

Decomposition: W = Wc + C, with C[n,k] = c[n, g(k)] a per-group constant
(shifted center chosen per group to minimize fp8 rounding error of Wc plus
the x-rounding coupling). Wc is scaled per output row by alpha_n in [1,2)
(chosen to minimize fp8 error); alpha is divided back out on the host.
The rank-8 group-constant term x @ C.T = s @ c.T (s = per-group sums of x)
is computed exactly on the host (a tiny 1.4-GFLOP GEMM) and added after
the device returns, so the device does ONLY the 16 k-pair fp8 DoubleRow
matmuls per output tile:
  out = fp16( (x8 @ (alpha*Wc8).T) * (1/t_m) ) / alpha_n + s @ c.T
All rounding (fp8/fp16) happens on host; device only does f32-accumulated
matmuls and an fp16 evict, so numerics are deterministic.

Shapes (hardcoded): x [4,2048,4096] f32 -> [8192,4096]; weight_packed
[88064,256] u8; out [4,2048,11008] f32. Per core: 1376 out-features.
"""

import numpy as np
import ml_dtypes

OUT_FEATURES = 11008
IN_FEATURES = 4096
GROUP_SIZE = 512
GPR = IN_FEATURES // GROUP_SIZE            # 8 groups per row
N_CORES = 8
N_SHARD = OUT_FEATURES // N_CORES          # 1376
M_TOTAL = 8192
M_TILE = 128
N_M_TILES = M_TOTAL // M_TILE              # 64
K_TILES = IN_FEATURES // 128               # 32
K_PAIRS = K_TILES // 2                     # 16
# processing order: the narrow slice second, so the weight tile needed
# earliest after slice 0 is the small one (better DMA margin in block 0)
N_SLICES = [(0, 512), (1024, 352), (512, 512)]

F8 = ml_dtypes.float8_e4m3
F16 = np.float16

_compiled = {}


# build-time knobs (sim-tuned)
OUT_ENGINE = "sync"  # engine queue for per-slice output DMAs
BLK = 16               # m-tiles per phase block
WARM_N = 44            # PE p-state warmup matmuls before real work


def _build():
    import concourse.bass as bass
    import concourse.mybir as mybir
    import concourse.tile as tile
    from concourse import bacc

    nc = bacc.Bacc(None, target_bir_lowering=False)
    f32, f16, fp8 = mybir.dt.float32, mybir.dt.float16, mybir.dt.float8e4
    DR = mybir.MatmulPerfMode.DoubleRow

    xt_in = nc.declare_dram_parameter("xt8", [N_M_TILES, 128, K_TILES, M_TILE], fp8, isOutput=False)
    wt_ins = [nc.declare_dram_parameter(f"wt8{j}", [128, K_TILES, cw], fp8, isOutput=False)
              for j, (c0, cw) in enumerate(N_SLICES)]
    tmi_in = nc.declare_dram_parameter("tminv", [128, N_M_TILES], f32, isOutput=False)
    out_ext = nc.declare_dram_parameter("out", [M_TOTAL, N_SHARD], f16, isOutput=True)

    out_eng = {"gpsimd": nc.gpsimd, "scalar": nc.scalar, "sync": nc.sync}[OUT_ENGINE]

    with tile.TileContext(nc) as tc:
        with (
            tc.tile_pool(name="wt", bufs=1) as wtp,
            tc.tile_pool(name="ext", bufs=1) as exp_,
            tc.tile_pool(name="xt", bufs=BLK + 1) as xtp,
            tc.tile_pool(name="osb", bufs=BLK + 1) as osp,
            tc.tile_pool(name="ps", bufs=7, space="PSUM") as psp,
        ):
            wts = [wtp.tile([128, K_TILES, cw], fp8, tag=f"WT{j}", name=f"wt{j}")
                   for j, (c0, cw) in enumerate(N_SLICES)]
            tmi = exp_.tile([128, N_M_TILES], f32, tag="TMI")

            if WARM_N:
                warm = exp_.tile([128, 2, 512], fp8, tag="WARM")
                nc.any.memset(warm, 0)
                wps = psp.tile([128, 512], f32, tag="ps", name="warm_ps")
                for w in range(WARM_N):
                    nc.tensor.matmul(wps, warm[:, :, 0:128], warm,
                                     start=True, stop=True, perf_mode=DR)

            for blk in range(N_M_TILES // BLK):
                xts, osbs = [], []
                for d in range(BLK):
                    xp = xtp.tile([128, K_TILES, M_TILE], fp8, tag="xt", name=f"xt_{blk}_{d}")
                    nc.sync.dma_start(xp, xt_in[blk * BLK + d, :, :, :])
                    xts.append(xp)
                    if blk == 0 and d == 0:
                        # slice-0 weights right after xt0: they gate the
                        # first chain
                        nc.sync.dma_start(wts[0], wt_ins[0][:, :, :])
                        nc.sync.dma_start(tmi, tmi_in[:, :])
                if blk == 0:
                    # slice 1/2 weights behind the full xt stream: the xt
                    # feed has no slack for mid-stream inserts, and these
                    # arrive well before their phases start
                    nc.sync.dma_start(wts[1], wt_ins[1][:, :, :])
                    nc.sync.dma_start(wts[2], wt_ins[2][:, :, :])
                for i in range(BLK):
                    osb_t = osp.tile([128, N_SHARD], f16, tag="osb", name=f"osb_{blk}_{i}")
                    osbs.append(osb_t)

                # last block ends on the narrow slice: shorter final
                # evict + output DMA on the critical tail
                jorder = [0, 1, 2] if blk < N_M_TILES // BLK - 1 else [0, 2, 1]
                for jo, j in enumerate(jorder):
                    c0, cw = N_SLICES[j]
                    for i in range(BLK):
                        mt = blk * BLK + i
                        m0 = mt * M_TILE
                        xtv = xts[i]
                        ps = psp.tile([128, 512], f32, tag="ps", name=f"ps{mt}_{j}")
                        for p in range(K_PAIRS):
                            nc.tensor.matmul(ps[:, :cw], xtv[:, 2 * p:2 * p + 2, :],
                                             wts[j][:, 2 * p:2 * p + 2, :],
                                             start=(p == 0), stop=(p == K_PAIRS - 1),
                                             perf_mode=DR)
                        nc.vector.tensor_scalar(osbs[i][:, c0:c0 + cw], ps[:, :cw],
                                                tmi[:, mt:mt + 1], None,
                                                mybir.AluOpType.mult)
                        if blk == 0:
                            # block 0's DMA-pool head window is fully booked
                            # with x tiles + weights; a per-slice output here
                            # becomes ready mid-stream and steals pool slots
                            # from the xt feed. One full-row DMA after the
                            # last phase keeps the head window clean.
                            if jo == 2:
                                out_eng.dma_start(out_ext[m0:m0 + M_TILE, :], osbs[i])
                        else:
                            out_eng.dma_start(out_ext[m0:m0 + M_TILE, c0:c0 + cw],
                                              osbs[i][:, c0:c0 + cw])

    nc.finalize()
    return nc


def _fp8r(a):
    return np.asarray(a, np.float32).astype(F8)


def _fp8_bits(x):
    """e4m3 RNE rounding via bit ops (fast scan path; matches ml_dtypes)."""
    x = np.asarray(x, np.float32)
    u = x.view(np.uint32)
    exp = (u >> 23) & 0xFF
    add = np.uint32(0x0007FFFF) + ((u >> np.uint32(20)) & np.uint32(1))
    xn = ((u + add) & np.uint32(0xFFF00000)).view(np.float32)
    q = np.float32(2.0 ** -9)
    xd = np.round(x / q) * q
    return np.where(exp >= 121, xn, xd).astype(np.float32)


def _row_scales(xf):
    """Per-row fp8 mantissa scale: best of 48 candidates over one octave,
    scored on a k-subsample."""
    sub = xf[:, ::2]
    best_err = None
    best_t = np.ones(xf.shape[0], np.float32)
    for t in (2.0 ** (np.arange(48) / 48.0)).astype(np.float32):
        e = _fp8_bits(sub * t) / t - sub
        err = (e.astype(np.float64) ** 2).sum(1)
        if best_err is None:
            best_err, best_t = err, np.full(xf.shape[0], t, np.float32)
        else:
            m = err < best_err
            best_err = np.where(m, err, best_err)
            best_t = np.where(m, t, best_t)
    return best_t


def _prep_weights(wp_u8, rng_f16, mn_f16, x_gamma):
    """Per-group shifted centers + per-row alpha scale (coordinate descent).

    Returns Wc8 fp8 [OUT, IN] (alpha-scaled), c [OUT, GPR] f64 (unscaled),
    alpha [OUT] f32.
    """
    lo = (wp_u8 & 15).astype(np.int8)
    hi = ((wp_u8 >> 4) & 15).astype(np.int8)
    q = np.stack([lo, hi], -1).reshape(-1, GROUP_SIZE)      # [NG, 512]
    ngrp = q.shape[0]
    rngf = np.asarray(rng_f16, np.float64)
    mnf = np.asarray(mn_f16, np.float64)

    counts = np.zeros((ngrp, 16), np.int32)
    for v in range(16):
        counts[:, v] = (q == v).sum(1)
    qm = (counts * np.arange(16)).sum(1) / GROUP_SIZE

    vals = np.arange(16, dtype=np.float64)

    def scan_d(alpha_row, dvals, d_base=None):
        best_J = None
        best_d = np.zeros(ngrp)
        for d in dvals:
            center = qm + d + (d_base if d_base is not None else 0.0)
            v = (vals[None, :] - center[:, None]) / 15.0 * rngf[:, None]
            va = (v * alpha_row[:, None]).astype(np.float32)
            e2 = ((_fp8r(va).astype(np.float32) - va) / alpha_row[:, None].astype(np.float32)) ** 2
            J = (counts * (e2 + x_gamma * v ** 2)).sum(1)
            if best_J is None:
                best_J = J
                best_d = (d_base if d_base is not None else 0.0) + np.full(ngrp, d)
            else:
                m = J < best_J
                best_J = np.where(m, J, best_J)
                best_d = np.where(m, (d_base if d_base is not None else 0.0) + d, best_d)
        return best_d, best_J

    def scan_a(d_cur, alphas):
        center = qm + d_cur
        v = (vals[None, :] - center[:, None]) / 15.0 * rngf[:, None]
        Jrow_best = None
        a_best = None
        for a in alphas:
            va = (v * a).astype(np.float32)
            e2 = ((_fp8r(va).astype(np.float32) - va) / np.float32(a)) ** 2
            J = (counts * (e2 + x_gamma * v ** 2)).sum(1).reshape(-1, GPR).sum(1)
            if Jrow_best is None:
                Jrow_best, a_best = J, np.full(len(J), a)
            else:
                m = J < Jrow_best
                Jrow_best = np.where(m, J, Jrow_best)
                a_best = np.where(m, a, a_best)
        return a_best

    ones = np.ones(ngrp)
    d0, _ = scan_d(ones, np.linspace(-2.0, 2.0, 33))
    d1, _ = scan_d(ones, np.linspace(-0.25, 0.25, 9), d_base=d0)
    a1 = scan_a(d1, 2.0 ** (np.arange(16) / 16.0))
    a_row = np.repeat(a1, GPR)
    d2, _ = scan_d(a_row, np.linspace(-2.0, 2.0, 33))
    d3, _ = scan_d(a_row, np.linspace(-0.25, 0.25, 9), d_base=d2)
    a2 = scan_a(d3, 2.0 ** (np.arange(24) / 24.0))
    a_row = np.repeat(a2, GPR)
    d4, _ = scan_d(a_row, np.linspace(-0.375, 0.375, 13), d_base=d3)
    d5, _ = scan_d(a_row, np.linspace(-0.0625, 0.0625, 5), d_base=d4)
    a3 = scan_a(d5, 2.0 ** (np.arange(24) / 24.0))

    center = qm + d5
    alpha = np.repeat(a3, GPR)
    a2 = a3
    c = (center / 15.0) * rngf + mnf                        # [NG] exact consts
    Wc = (q.astype(np.float32) - center[:, None].astype(np.float32)) \
        / np.float32(15.0) * rngf[:, None].astype(np.float32)
    Wc8 = _fp8r(Wc * alpha[:, None].astype(np.float32)).reshape(OUT_FEATURES, IN_FEATURES)
    return Wc8, c.reshape(OUT_FEATURES, GPR), a2.astype(np.float32)


def _host_prep(x, weight_packed, weight_range, weight_min):
    """All host-side quantization; returns device arrays + host-side terms."""
    xf = np.ascontiguousarray(np.asarray(x, dtype=np.float32).reshape(M_TOTAL, IN_FEATURES))
    wp = np.asarray(weight_packed).astype(np.uint8)

    lam = 0.000663  # E[dx^2]/E[x^2] for fp8 e4m3 with the 32-cand row scales
    Wc8, c, alpha = _prep_weights(wp, weight_range, weight_min, x_gamma=lam)

    tm = _row_scales(xf)[:, None]                           # [8192, 1]
    x8 = _fp8r(xf * tm)                                     # [8192, 4096] fp8

    # exact rank-8 group-constant term, added on host after the device GEMM
    s = xf.astype(np.float64).reshape(M_TOTAL, GPR, GROUP_SIZE).sum(-1)
    corr = (s @ c.T).astype(np.float32)                     # [8192, OUT]

    tminv = np.ascontiguousarray(
        (1.0 / tm[:, 0]).astype(np.float32).reshape(N_M_TILES, M_TILE).T)
    return dict(x8=x8, Wc8=Wc8, corr=corr, tm=tm, tminv=tminv, alpha=alpha)


def _device_arrays(hp):
    """Pack host-prep outputs into the device input layout (per-core maps)."""
    x8, Wc8 = hp["x8"], hp["Wc8"]
    xt8 = np.ascontiguousarray(
        x8.reshape(N_M_TILES, M_TILE, K_TILES, 128).transpose(0, 3, 2, 1))

    in_maps = []
    for core in range(N_CORES):
        n0 = core * N_SHARD
        wt8 = Wc8[n0:n0 + N_SHARD].reshape(N_SHARD, K_TILES, 128).transpose(2, 1, 0)
        imap = {"xt8": xt8, "tminv": hp["tminv"]}
        for j, (c0, cw) in enumerate(N_SLICES):
            imap[f"wt8{j}"] = np.ascontiguousarray(wt8[:, :, c0:c0 + cw])
        in_maps.append(imap)
    return in_maps


def kernel(x, weight_packed, weight_range, weight_min):
    from concourse.bass_utils import run_bass_kernel_spmd

    if "nc" not in _compiled:
        _compiled["nc"] = _build()
    nc = _compiled["nc"]

    hp = _host_prep(x, weight_packed, weight_range, weight_min)
    in_maps = _device_arrays(hp)

    res = run_bass_kernel_spmd(nc, in_maps, core_ids=list(range(N_CORES)))
    _compiled["last_res"] = res
    ainv = (1.0 / hp["alpha"]).astype(np.float32)
    shards = [np.asarray(res.results[core]["out"]).astype(np.float32)
              for core in range(N_CORES)]
    full = np.concatenate(shards, axis=1) * ainv[None, :] + hp["corr"]
    return full.reshape(4, 2048, OUT_FEATURES).astype(np.float32)


# revision 39
# speedup vs baseline: 1.0004x; 1.0004x over previous
"""4-bit group-quantized linear via fp8 DoubleRow matmul, column-parallel on 8 cores.

out = x @ W.T with W[n,k] = (q/15)*range[n,g] + min[n,g], groups of 512 k.

Decomposition: W = Wc + C, with C[n,k] = c[n, g(k)] a per-group constant
(shifted center chosen per group to minimize fp8 rounding error of Wc plus
the x-rounding coupling). Wc is scaled per output row by alpha_n in [1,2)
(chosen to minimize fp8 error); alpha is divided back out on the host.
The rank-8 group-constant term x @ C.T = s @ c.T (s = per-group sums of x)
is computed exactly on the host (a tiny 1.4-GFLOP GEMM) and added after
the device returns, so the device does ONLY the 16 k-pair fp8 DoubleRow
matmuls per output tile:
  out = fp16( (x8 @ (alpha*Wc8).T) * (1/t_m) ) / alpha_n + s @ c.T
All rounding (fp8/fp16) happens on host; device only does f32-accumulated
matmuls and an fp16 evict, so numerics are deterministic.

Shapes (hardcoded): x [4,2048,4096] f32 -> [8192,4096]; weight_packed
[88064,256] u8; out [4,2048,11008] f32. Per core: 1376 out-features.
"""

import numpy as np
import ml_dtypes

OUT_FEATURES = 11008
IN_FEATURES = 4096
GROUP_SIZE = 512
GPR = IN_FEATURES // GROUP_SIZE            # 8 groups per row
N_CORES = 8
N_SHARD = OUT_FEATURES // N_CORES          # 1376
M_TOTAL = 8192
M_TILE = 128
N_M_TILES = M_TOTAL // M_TILE              # 64
K_TILES = IN_FEATURES // 128               # 32
K_PAIRS = K_TILES // 2                     # 16
# processing order: the narrow slice second, so the weight tile needed
# earliest after slice 0 is the small one (better DMA margin in block 0)
N_SLICES = [(0, 512), (1024, 352), (512, 512)]

F8 = ml_dtypes.float8_e4m3
F16 = np.float16

_compiled = {}


# build-time knobs (sim-tuned)
OUT_ENGINE = "sync"  # engine queue for per-slice output DMAs
BLK = 16               # m-tiles per phase block
WARM_N = 44            # PE p-state warmup matmuls before real work


def _build():
    import concourse.bass as bass
    import concourse.mybir as mybir
    import concourse.tile as tile
    from concourse import bacc

    nc = bacc.Bacc(None, target_bir_lowering=False)
    f32, f16, fp8 = mybir.dt.float32, mybir.dt.float16, mybir.dt.float8e4
    DR = mybir.MatmulPerfMode.DoubleRow

    xt_in = nc.declare_dram_parameter("xt8", [N_M_TILES, 128, K_TILES, M_TILE], fp8, isOutput=False)
    wt_ins = [nc.declare_dram_parameter(f"wt8{j}", [128, K_TILES, cw], fp8, isOutput=False)
              for j, (c0, cw) in enumerate(N_SLICES)]
    tmi_in = nc.declare_dram_parameter("tminv", [128, N_M_TILES], f32, isOutput=False)
    out_ext = nc.declare_dram_parameter("out", [M_TOTAL, N_SHARD], f16, isOutput=True)

    out_eng = {"gpsimd": nc.gpsimd, "scalar": nc.scalar, "sync": nc.sync}[OUT_ENGINE]

    with tile.TileContext(nc) as tc:
        with (
            tc.tile_pool(name="wt", bufs=1) as wtp,
            tc.tile_pool(name="ext", bufs=1) as exp_,
            tc.tile_pool(name="xt", bufs=BLK + 1) as xtp,
            tc.tile_pool(name="osb", bufs=BLK + 1) as osp,
            tc.tile_pool(name="ps", bufs=7, space="PSUM") as psp,
        ):
            wt0a = wtp.tile([128, K_TILES // 2, N_SLICES[0][1]], fp8, tag="WT0a")
            wt0b = wtp.tile([128, K_TILES // 2, N_SLICES[0][1]], fp8, tag="WT0b")
            wts = [(wt0a, wt0b)] + \
                  [wtp.tile([128, K_TILES, cw], fp8, tag=f"WT{j}", name=f"wt{j}")
                   for j, (c0, cw) in list(enumerate(N_SLICES))[1:]]

            def wslice(j, p):
                # [128, 2, cw] view of weight slice j at kt-pair p
                if j == 0:
                    half = wts[0][p // 8]
                    return half[:, 2 * (p % 8):2 * (p % 8) + 2, :]
                return wts[j][:, 2 * p:2 * p + 2, :]

            tmi = exp_.tile([128, N_M_TILES], f32, tag="TMI")

            if WARM_N:
                warm = exp_.tile([128, 2, 512], fp8, tag="WARM")
                nc.any.memset(warm, 0)
                wps = psp.tile([128, 512], f32, tag="ps", name="warm_ps")
                for w in range(WARM_N):
                    nc.tensor.matmul(wps, warm[:, :, 0:128], warm,
                                     start=True, stop=True, perf_mode=DR)

            for blk in range(N_M_TILES // BLK):
                xts, osbs = [], []
                for d in range(BLK):
                    xp = xtp.tile([128, K_TILES, M_TILE], fp8, tag="xt", name=f"xt_{blk}_{d}")
                    nc.sync.dma_start(xp, xt_in[blk * BLK + d, :, :, :])
                    xts.append(xp)
                    if blk == 0 and d == 0:
                        # slice-0 weights right after xt0: they gate the
                        # first chain (two halves so it can start early)
                        nc.sync.dma_start(wts[0][0], wt_ins[0][:, 0:K_TILES // 2, :])
                        nc.sync.dma_start(wts[0][1], wt_ins[0][:, K_TILES // 2:, :])
                        nc.sync.dma_start(tmi, tmi_in[:, :])
                if blk == 0:
                    # slice 1/2 weights behind the full xt stream: the xt
                    # feed has no slack for mid-stream inserts, and these
                    # arrive well before their phases start
                    nc.sync.dma_start(wts[1], wt_ins[1][:, :, :])
                    nc.sync.dma_start(wts[2], wt_ins[2][:, :, :])
                for i in range(BLK):
                    osb_t = osp.tile([128, N_SHARD], f16, tag="osb", name=f"osb_{blk}_{i}")
                    osbs.append(osb_t)

                # last block ends on the narrow slice: shorter final
                # evict + output DMA on the critical tail
                jorder = [0, 1, 2] if blk < N_M_TILES // BLK - 1 else [0, 2, 1]
                for jo, j in enumerate(jorder):
                    c0, cw = N_SLICES[j]
                    for i in range(BLK):
                        mt = blk * BLK + i
                        m0 = mt * M_TILE
                        xtv = xts[i]
                        ps = psp.tile([128, 512], f32, tag="ps", name=f"ps{mt}_{j}")
                        for p in range(K_PAIRS):
                            nc.tensor.matmul(ps[:, :cw], xtv[:, 2 * p:2 * p + 2, :],
                                             wslice(j, p),
                                             start=(p == 0), stop=(p == K_PAIRS - 1),
                                             perf_mode=DR)
                        nc.vector.tensor_scalar(osbs[i][:, c0:c0 + cw], ps[:, :cw],
                                                tmi[:, mt:mt + 1], None,
                                                mybir.AluOpType.mult)
                        if blk == 0:
                            # block 0's DMA-pool head window is fully booked
                            # with x tiles + weights; a per-slice output here
                            # becomes ready mid-stream and steals pool slots
                            # from the xt feed. One full-row DMA after the
                            # last phase keeps the head window clean.
                            if jo == 2:
                                out_eng.dma_start(out_ext[m0:m0 + M_TILE, :], osbs[i])
                        else:
                            out_eng.dma_start(out_ext[m0:m0 + M_TILE, c0:c0 + cw],
                                              osbs[i][:, c0:c0 + cw])

    nc.finalize()
    return nc


def _fp8r(a):
    return np.asarray(a, np.float32).astype(F8)


def _fp8_bits(x):
    """e4m3 RNE rounding via bit ops (fast scan path; matches ml_dtypes)."""
    x = np.asarray(x, np.float32)
    u = x.view(np.uint32)
    exp = (u >> 23) & 0xFF
    add = np.uint32(0x0007FFFF) + ((u >> np.uint32(20)) & np.uint32(1))
    xn = ((u + add) & np.uint32(0xFFF00000)).view(np.float32)
    q = np.float32(2.0 ** -9)
    xd = np.round(x / q) * q
    return np.where(exp >= 121, xn, xd).astype(np.float32)


def _row_scales(xf):
    """Per-row fp8 mantissa scale: best of 48 candidates over one octave,
    scored on a k-subsample."""
    sub = xf[:, ::2]
    best_err = None
    best_t = np.ones(xf.shape[0], np.float32)
    for t in (2.0 ** (np.arange(48) / 48.0)).astype(np.float32):
        e = _fp8_bits(sub * t) / t - sub
        err = (e.astype(np.float64) ** 2).sum(1)
        if best_err is None:
            best_err, best_t = err, np.full(xf.shape[0], t, np.float32)
        else:
            m = err < best_err
            best_err = np.where(m, err, best_err)
            best_t = np.where(m, t, best_t)
    return best_t


def _prep_weights(wp_u8, rng_f16, mn_f16, x_gamma):
    """Per-group shifted centers + per-row alpha scale (coordinate descent).

    Returns Wc8 fp8 [OUT, IN] (alpha-scaled), c [OUT, GPR] f64 (unscaled),
    alpha [OUT] f32.
    """
    lo = (wp_u8 & 15).astype(np.int8)
    hi = ((wp_u8 >> 4) & 15).astype(np.int8)
    q = np.stack([lo, hi], -1).reshape(-1, GROUP_SIZE)      # [NG, 512]
    ngrp = q.shape[0]
    rngf = np.asarray(rng_f16, np.float64)
    mnf = np.asarray(mn_f16, np.float64)

    counts = np.zeros((ngrp, 16), np.int32)
    for v in range(16):
        counts[:, v] = (q == v).sum(1)
    qm = (counts * np.arange(16)).sum(1) / GROUP_SIZE

    vals = np.arange(16, dtype=np.float64)

    def scan_d(alpha_row, dvals, d_base=None):
        best_J = None
        best_d = np.zeros(ngrp)
        for d in dvals:
            center = qm + d + (d_base if d_base is not None else 0.0)
            v = (vals[None, :] - center[:, None]) / 15.0 * rngf[:, None]
            va = (v * alpha_row[:, None]).astype(np.float32)
            e2 = ((_fp8r(va).astype(np.float32) - va) / alpha_row[:, None].astype(np.float32)) ** 2
            J = (counts * (e2 + x_gamma * v ** 2)).sum(1)
            if best_J is None:
                best_J = J
                best_d = (d_base if d_base is not None else 0.0) + np.full(ngrp, d)
            else:
                m = J < best_J
                best_J = np.where(m, J, best_J)
                best_d = np.where(m, (d_base if d_base is not None else 0.0) + d, best_d)
        return best_d, best_J

    def scan_a(d_cur, alphas):
        center = qm + d_cur
        v = (vals[None, :] - center[:, None]) / 15.0 * rngf[:, None]
        Jrow_best = None
        a_best = None
        for a in alphas:
            va = (v * a).astype(np.float32)
            e2 = ((_fp8r(va).astype(np.float32) - va) / np.float32(a)) ** 2
            J = (counts * (e2 + x_gamma * v ** 2)).sum(1).reshape(-1, GPR).sum(1)
            if Jrow_best is None:
                Jrow_best, a_best = J, np.full(len(J), a)
            else:
                m = J < Jrow_best
                Jrow_best = np.where(m, J, Jrow_best)
                a_best = np.where(m, a, a_best)
        return a_best

    ones = np.ones(ngrp)
    d0, _ = scan_d(ones, np.linspace(-2.0, 2.0, 33))
    d1, _ = scan_d(ones, np.linspace(-0.25, 0.25, 9), d_base=d0)
    a1 = scan_a(d1, 2.0 ** (np.arange(16) / 16.0))
    a_row = np.repeat(a1, GPR)
    d2, _ = scan_d(a_row, np.linspace(-2.0, 2.0, 33))
    d3, _ = scan_d(a_row, np.linspace(-0.25, 0.25, 9), d_base=d2)
    a2 = scan_a(d3, 2.0 ** (np.arange(24) / 24.0))
    a_row = np.repeat(a2, GPR)
    d4, _ = scan_d(a_row, np.linspace(-0.375, 0.375, 13), d_base=d3)
    d5, _ = scan_d(a_row, np.linspace(-0.0625, 0.0625, 5), d_base=d4)
    a3 = scan_a(d5, 2.0 ** (np.arange(24) / 24.0))

    center = qm + d5
    alpha = np.repeat(a3, GPR)
    a2 = a3
    c = (center / 15.0) * rngf + mnf                        # [NG] exact consts
    Wc = (q.astype(np.float32) - center[:, None].astype(np.float32)) \
        / np.float32(15.0) * rngf[:, None].astype(np.float32)
    Wc8 = _fp8r(Wc * alpha[:, None].astype(np.float32)).reshape(OUT_FEATURES, IN_FEATURES)
    return Wc8, c.reshape(OUT_FEATURES, GPR), a2.astype(np.float32)


def _host_prep(x, weight_packed, weight_range, weight_min):
    """All host-side quantization; returns device arrays + host-side terms."""
    xf = np.ascontiguousarray(np.asarray(x, dtype=np.float32).reshape(M_TOTAL, IN_FEATURES))
    wp = np.asarray(weight_packed).astype(np.uint8)

    lam = 0.000663  # E[dx^2]/E[x^2] for fp8 e4m3 with the 32-cand row scales
    Wc8, c, alpha = _prep_weights(wp, weight_range, weight_min, x_gamma=lam)

    tm = _row_scales(xf)[:, None]                           # [8192, 1]
    x8 = _fp8r(xf * tm)                                     # [8192, 4096] fp8

    # exact rank-8 group-constant term, added on host after the device GEMM
    s = xf.astype(np.float64).reshape(M_TOTAL, GPR, GROUP_SIZE).sum(-1)
    corr = (s @ c.T).astype(np.float32)                     # [8192, OUT]

    tminv = np.ascontiguousarray(
        (1.0 / tm[:, 0]).astype(np.float32).reshape(N_M_TILES, M_TILE).T)
    return dict(x8=x8, Wc8=Wc8, corr=corr, tm=tm, tminv=tminv, alpha=alpha)


def _device_arrays(hp):
    """Pack host-prep outputs into the device input layout (per-core maps)."""
    x8, Wc8 = hp["x8"], hp["Wc8"]
    xt8 = np.ascontiguousarray(
        x8.reshape(N_M_TILES, M_TILE, K_TILES, 128).transpose(0, 3, 2, 1))

    in_maps = []
    for core in range(N_CORES):
        n0 = core * N_SHARD
        wt8 = Wc8[n0:n0 + N_SHARD].reshape(N_SHARD, K_TILES, 128).transpose(2, 1, 0)
        imap = {"xt8": xt8, "tminv": hp["tminv"]}
        for j, (c0, cw) in enumerate(N_SLICES):
            imap[f"wt8{j}"] = np.ascontiguousarray(wt8[:, :, c0:c0 + cw])
        in_maps.append(imap)
    return in_maps


def kernel(x, weight_packed, weight_range, weight_min):
    from concourse.bass_utils import run_bass_kernel_spmd

    if "nc" not in _compiled:
        _compiled["nc"] = _build()
    nc = _compiled["nc"]

    hp = _host_prep(x, weight_packed, weight_range, weight_min)
    in_maps = _device_arrays(hp)

    res = run_bass_kernel_spmd(nc, in_maps, core_ids=list(range(N_CORES)))
    _compiled["last_res"] = res
    ainv = (1.0 / hp["alpha"]).astype(np.float32)
    shards = [np.asarray(res.results[core]["out"]).astype(np.float32)
              for core in range(N_CORES)]
    full = np.concatenate(shards, axis=1) * ainv[None, :] + hp["corr"]
    return full.reshape(4, 2048, OUT_FEATURES).astype(np.float32)
